# revision 26
# baseline (speedup 1.0000x reference)
"""CodaPrompt kernel for Trainium2 (Bass/Tile) on 8 NeuronCores.

Math (reference):
    a[e,b,k,:] = x[b,:] * As[e,k,:]
    q = a / max(||a||_2, eps)        (normalize over d)
    nK = Ks / max(||Ks||_2, eps)
    aq[e,b,k] = <q[e,b,k,:], nK[e,k,:]>
    P_[e,b,l,:] = sum_k aq[e,b,k] * Ps[e,k,l,:]
    out = stack([P_[:,:, :L/2], P_[:,:, L/2:]])   # [2, E, B, L/2, D]

Sharding: SSPLIT L-slices x (8/SSPLIT) batch-slices (default 4x2). Splitting
the output L-axis (the stack axis) cuts each core's Ps load to 1/SSPLIT vs
pure batch-parallel at identical arithmetic — the kernel is DMA-bound and
the output (31.5MB/core) is fixed, so input bytes are the only lever
(default config: 3.1MB Ps + 3.1MB x + 3.1MB weights vs 16.2MB for pure
batch-parallel). Each core computes the full cosine-weight stage (aq) for
its batch slice (duplicated across L-slices; PE has slack).

Device-side formulation (per core: batch slice of BC rows, one L-slice):
    num[e,k,b] = sum_d (As*nK)[e,k,d] * x[b,d]        -> matmul, contraction over d
    den2[e,k,b] = sum_d (As*As)[e,k,d] * x2[b,d]      -> matmul (x2 on device)
    aq[e,k,b] = num * rsqrt(den2)                      (ACT sqrt + DVE recip + mul)
    P_half[b, (l d)] = aq[e,:,b].T @ Ps[e, :, half]    -> matmul, contraction over k

Host prep is O(E*K*D) pool preprocessing (normalize Ks, fuse/transpose
weights, slice Ps halves) plus the x transpose; all O(B*...) FLOPs on device.
"""

import os
import sys
from contextlib import ExitStack

import numpy as np

if "/opt/trn_rl_repo" not in sys.path:
    sys.path.insert(0, "/opt/trn_rl_repo")

import concourse.mybir as mybir
from concourse import bacc, tile
from concourse.bass_utils import run_bass_kernel_spmd

B, D, E, K, L = 2048, 768, 5, 100, 8
NCORES = 8
SSPLIT = int(os.environ.get("CODA_SSPLIT", "4"))  # L-axis splits (2 or 4)
QSPLIT = NCORES // SSPLIT # batch splits
BC = B // QSPLIT          # batch rows per core
LH = L // SSPLIT          # l entries per core
DC = D // 128             # 6 contraction chunks of 128
NDH = LH * D              # P_ cols per core
NCHUNK = 512              # psum bank width in f32
NJ = NDH // NCHUNK        # n-chunks per core
MC = BC // 128            # output-partition chunks
NB = max(1, BC // 512)    # moving-operand chunks for num/den (fp32 N<=512)
EPS = 1e-12

F32 = mybir.dt.float32
# "float32r" = single-pass reduced-precision fp32 matmul (full PE rate at
# N>=256), ~2.1e-4 scale-relative error; "float32" = exact, 4 cycles/row.
MM_DTYPE = os.environ.get("CODA_MM_DTYPE", "float32r")
MM_DT = getattr(mybir.dt, MM_DTYPE)
# Optional: carry the prompt pool (and the aq weights feeding the same
# matmul) in bf16 — halves the Ps DMA at ~2e-3 scale-relative error.
PS_DTYPE = os.environ.get("CODA_PS_DTYPE", MM_DTYPE)
PS_DT = getattr(mybir.dt, PS_DTYPE)


def _build_bass(repeat=1):
    # Bacc (not plain Bass): its finalize() runs move_matmul_waits_to_ldweights
    # + generate_event_semaphores, without which multi-dependency matmuls hit
    # walrus "Too many sync wait commands".
    # `repeat` replicates the whole compute body (timing instrumentation:
    # slope over repeat removes per-launch overhead); results are idempotent.
    nc = bacc.Bacc(None)

    # Matmul operands must be produced as MM_DT end-to-end (walrus verifies
    # fp32r consumers see fp32r producers). float32r is bit-identical to
    # float32 in DRAM, so host arrays stay np.float32 either way.
    xT_d = nc.declare_dram_parameter("xT", [D, BC], MM_DT, isOutput=False)
    w_d = nc.declare_dram_parameter("w12T", [D, 2, E, K], MM_DT, isOutput=False)
    ps_d = nc.declare_dram_parameter("ps", [E, K, NDH], PS_DT, isOutput=False)
    out_d = nc.declare_dram_parameter("out", [E, BC, LH, D], F32, isOutput=True)

    with ExitStack() as ctx:
        tc = ctx.enter_context(tile.TileContext(nc))
        const = ctx.enter_context(tc.tile_pool(name="const", bufs=1))
        psp = ctx.enter_context(tc.tile_pool(name="psp", bufs=E))
        smallp = ctx.enter_context(tc.tile_pool(name="smallp", bufs=2))
        resp = ctx.enter_context(tc.tile_pool(name="resp", bufs=4))
        # num/den psum tiles span ceil(BC*4B/2KB) banks; keep total <= 8.
        pndp = ctx.enter_context(
            tc.tile_pool(name="pndp", bufs=(2 if BC <= 512 else 1), space="PSUM")
        )
        ppp = ctx.enter_context(tc.tile_pool(name="ppp", bufs=4, space="PSUM"))

        # Resident operands: x quarter (transposed) and the fused W1=As*nK /
        # W2=As^2 weight block, chunked to 128 partitions. Per-chunk loads so
        # the first num/den matmuls start as soon as their own d-chunk lands.
        # x^2 is computed on-device (saves its DMA).
        xT_r = xT_d[:].rearrange("(c p) b -> p c b", p=128)
        w_r = w_d[:].rearrange("(c p) t e k -> p c t e k", p=128)
        xs = const.tile([128, DC, BC], MM_DT, name="xs", tag="xs")
        x2s = const.tile([128, DC, BC], MM_DT, name="x2s", tag="x2s")
        ws = const.tile([128, DC, 2, E, K], MM_DT, name="ws", tag="ws")
        for c in range(DC):
            nc.sync.dma_start(ws[:, c], w_r[:, c])
            nc.sync.dma_start(xs[:, c], xT_r[:, c])
            nc.vector.tensor_mul(x2s[:, c], xs[:, c], xs[:, c])

        for _ in range(repeat):
            # All pool loads issue upfront (own slots, bufs=E) so no load
            # ever queues behind output stores in a DMA FIFO.
            psts = []
            for e in range(E):
                pst = psp.tile([K, NDH], PS_DT, name="pst", tag="ps")
                nc.sync.dma_start(pst[:], ps_d[e])
                psts.append(pst)
            # Per e: cosine weights aq[e] (PE d-contraction), then its P_
            # blocks — interleaved trace order so output stores start flowing
            # as soon as the first aq is ready (keeps DMA busy once the small
            # input loads finish).
            for e in range(E):
                num = pndp.tile([K, BC], F32, name="num", tag="num")
                den = pndp.tile([K, BC], F32, name="den", tag="den")
                for nb in range(NB):
                    bsl = slice(nb * 512, min((nb + 1) * 512, BC))
                    for c in range(DC):
                        nc.tensor.matmul(
                            num[:, bsl],
                            ws[:, c, 0, e, :],
                            xs[:, c, bsl],
                            start=(c == 0),
                            stop=(c == DC - 1),
                        )
                    for c in range(DC):
                        nc.tensor.matmul(
                            den[:, bsl],
                            ws[:, c, 1, e, :],
                            x2s[:, c, bsl],
                            start=(c == 0),
                            stop=(c == DC - 1),
                        )
                # aq = num / sqrt(den2)   (den2 >> eps^2 for this regime)
                sden = smallp.tile([K, BC], F32, name="sden", tag="sden")
                nc.scalar.sqrt(sden[:], den[:])
                rden = smallp.tile([K, BC], F32, name="rden", tag="rden")
                nc.vector.reciprocal(rden[:], sden[:])
                aq = smallp.tile([K, BC], PS_DT, name="aq", tag="aq", bufs=2)
                nc.vector.tensor_mul(aq[:], num[:], rden[:])
                pst = psts[e]
                # Store groups: pairs of psum chunks when NJ is even, else
                # one group of NJ (small SBUF slots, early drain, short tail).
                groups = [2] * (NJ // 2) if NJ % 2 == 0 else [NJ]
                for m in range(MC):
                    j0 = 0
                    for glen in groups:
                        res = resp.tile(
                            [128, max(groups) * NCHUNK], F32, name="res", tag="res"
                        )[:, : glen * NCHUNK]
                        for jj in range(glen):
                            j = j0 + jj
                            pp = ppp.tile([128, NCHUNK], F32, name="pp", tag="pp")
                            nc.tensor.matmul(
                                pp[:],
                                aq[:, m * 128 : (m + 1) * 128],
                                pst[:, j * NCHUNK : (j + 1) * NCHUNK],
                                start=True,
                                stop=True,
                            )
                            dst = res[:, jj * NCHUNK : (jj + 1) * NCHUNK]
                            if j % 2 == 0:
                                nc.vector.tensor_copy(dst, pp[:])
                            else:
                                nc.scalar.copy(dst, pp[:])
                        out_ap = out_d[e, m * 128 : (m + 1) * 128, :, :].rearrange(
                            "b l d -> b (l d)"
                        )
                        nc.sync.dma_start(
                            out_ap[:, j0 * NCHUNK : (j0 + glen) * NCHUNK],
                            res[:],
                        )
                        j0 += glen

    if not nc.is_finalized():
        nc.finalize()
    return nc


_NC_CACHE = None


def _get_nc():
    global _NC_CACHE
    if _NC_CACHE is None:
        _NC_CACHE = _build_bass()
    return _NC_CACHE


def _prep_inputs(x, Ks, As, Ps):
    x = np.asarray(x, dtype=np.float32)
    Ks = np.asarray(Ks, dtype=np.float32)
    As = np.asarray(As, dtype=np.float32)
    Ps = np.asarray(Ps, dtype=np.float32)

    nrm = np.sqrt(np.sum(Ks * Ks, axis=-1, keepdims=True))
    nK = Ks / np.maximum(nrm, EPS)
    w12T = np.empty((D, 2, E, K), dtype=np.float32)
    w12T[:, 0] = (As * nK).transpose(2, 0, 1)
    w12T[:, 1] = (As * As).transpose(2, 0, 1)

    ps_np = mybir.dt.np(PS_DT)
    ps_slices = [
        np.ascontiguousarray(
            Ps[:, :, si * LH : (si + 1) * LH, :].reshape(E, K, NDH)
        ).astype(ps_np, copy=False)
        for si in range(SSPLIT)
    ]
    xT = np.ascontiguousarray(x.T)          # [D, B]

    in_maps = []
    for c in range(NCORES):
        si, q = divmod(c, QSPLIT)
        in_maps.append(
            {
                "xT": np.ascontiguousarray(xT[:, q * BC : (q + 1) * BC]),
                "w12T": w12T,
                "ps": ps_slices[si],
            }
        )
    return in_maps


def _run(x, Ks, As, Ps, trace=False, **spmd_kwargs):
    nc = _get_nc()
    in_maps = _prep_inputs(x, Ks, As, Ps)
    res = run_bass_kernel_spmd(nc, in_maps, list(range(NCORES)), trace=trace, **spmd_kwargs)
    out = np.empty((2, E, B, L // 2, D), dtype=np.float32)
    for c in range(NCORES):
        si, q = divmod(c, QSPLIT)
        s, lp = divmod(si * LH, L // 2)
        out[s, :, q * BC : (q + 1) * BC, lp : lp + LH] = res.results[c]["out"]
    return out, res


def kernel(x, Ks, As, Ps):
    out, _ = _run(x, Ks, As, Ps, trace=False)
    return out
